# revision 25
# baseline (speedup 1.0000x reference)
"""
Bass/Trainium2 kernel for nn_Attention_72456098284196.

Attention module: QKV projections + partial rotary (first 32 of 64 head
channels, all heads) + softmax attention.  B=2, T=2048, C=1024, H=16, D=64.

Sharding: 8 NeuronCores = 2 batches x 4 head-groups (4 heads each).
Pure tensor/batch parallel -> no collectives; host slices inputs and
concatenates outputs.

Host prep: x / W are cast to bf16 and transposed; the rotary cos/sin
tables are precomputed from the position inputs ([128, T] bf16, one row
per channel of a 2-head c-tile; the splice sign is folded into the sin
table so the device-side splice is a plain partition pair-swap).

Device-side dataflow (per core, matmuls bf16 with fp32 PSUM accum):
  DMA: few large 128-partition transfers (x as two 2MB t-halves per
       tensor) so pair-0 projections start ~6us in.
  qT[c,t] = WqT.T @ xqT   (c = 4 heads x 64 ch, two 128-partition c-tiles)
  rotary:  rq = (q+b)*cos + (shuffle(q)+swap(b))*sin_signed on DVE
           (stream_shuffle pair-swap; passthrough channels cos=1,sin=0)
  scores^T[s,t] = rkT.T @ rqT per head, 64x64 PE tiling (two s-halves to
           disjoint col groups stream concurrently)
  expT = Exp(scores^T / 8) on ScalarE -> bf16.  ScalarE is the pacing
           engine (~1.14us per [128,1024] tile, 128 tiles = ~145us); the
           schedule keeps it fed from ~15us to the end:
             window it0: scores(it0) + q0/k0/v/q1/k1 projections backfill
             window itN: scores(itN) + AV(itN-1) backfill
  outT[d,t] accum over s of [v | 1].T @ expT  (M=65: row 64 accumulates
           the softmax denominator for free)
  normalize: recip(denominator row) via fast approx, broadcast 1->64
           partitions via a bf16 PE outer product, multiply, DMA out.
"""

import math
import sys

import numpy as np

if "/opt/trn_rl_repo" not in sys.path:
    sys.path.insert(0, "/opt/trn_rl_repo")

import concourse.bass as bass  # noqa: E402
import concourse.mybir as mybir  # noqa: E402
import concourse.tile as tile  # noqa: E402
from concourse import bacc  # noqa: E402
from concourse.bass_utils import run_bass_kernel_spmd  # noqa: E402

B, T, C = 2, 2048, 1024
NUM_HEADS = 16
HEAD_DIM = 64
N_CORES = 8
HEADS_PER_CORE = NUM_HEADS // (N_CORES // B)  # 4
CO = HEADS_PER_CORE * HEAD_DIM  # 256 out channels per core
N_ROT = 32  # rotated channels per head
MAX_WAVELENGTH = 8192.0

F32 = mybir.dt.float32
BF16 = mybir.dt.bfloat16
NPBF16 = mybir.dt.np(BF16)

P = 128  # partitions
TCH = 512  # matmul N chunk (1 PSUM bank)
KCH = C // P  # 8 contraction chunks
NCT = CO // P  # 2 c-tiles (each = 2 heads x 64)
NST = T // P  # 16 s tiles
SCALE = 1.0 / math.sqrt(HEAD_DIM)
TH = 1024  # attention t-half width / x DMA half width
XH = 2


def _inv_freq() -> np.ndarray:
    """[32] inverse frequencies (pairs repeated), matching the reference."""
    num_bands = N_ROT // 2  # 16
    freq = MAX_WAVELENGTH ** (
        2.0 / N_ROT * np.linspace(0.0, num_bands, num_bands, dtype=np.float64)
    )
    return np.repeat(1.0 / freq, 2)  # [32]


def _rot_tables(pos: np.ndarray) -> tuple[np.ndarray, np.ndarray]:
    """cos/sin tables [128, T] bf16 for a 2-head c-tile.

    Rows r in [0,32) and [64,96): rotary channels (cos/sin of pos*invf);
    other rows: cos=1, sin=0 (passthrough).  The splice negation is
    folded into sin: even channels get -sin so that
    rq = q*cos + pairswap(q)*sin_signed.
    """
    inv = _inv_freq()  # [32]
    rad = pos.astype(np.float64)[None, :] * inv[:, None]  # [32, T]
    cos32 = np.cos(rad)
    sin32 = np.sin(rad)
    sign = np.where(np.arange(N_ROT) % 2 == 0, -1.0, 1.0)[:, None]
    sin32 = sin32 * sign
    cos = np.zeros((P, pos.shape[0]), np.float64)
    sin = np.zeros((P, pos.shape[0]), np.float64)
    for o in (0, 64):
        cos[o : o + N_ROT] = cos32
        sin[o : o + N_ROT] = sin32
        cos[o + N_ROT : o + 64] = 1.0
    return cos.astype(NPBF16), sin.astype(NPBF16)


_SWAP_MASK = [i ^ 1 for i in range(32)]  # pair swap within each 32-quadrant


def build_bass() -> bass.Bass:
    nc = bacc.Bacc()

    # x / w are host-prepacked partition-major ([P, KCH, ...]) so each
    # DMA moves 128 x 16KB contiguous per-partition runs.
    xq_ext = [
        nc.declare_dram_parameter(f"xqT{h}", [P, KCH, TH], BF16, isOutput=False)
        for h in range(XH)
    ]
    xkv_ext = [
        nc.declare_dram_parameter(f"xkvT{h}", [P, KCH, TH], BF16, isOutput=False)
        for h in range(XH)
    ]
    wq_ext = nc.declare_dram_parameter("wqT", [P, KCH, CO], BF16, isOutput=False)
    wk_ext = nc.declare_dram_parameter("wkT", [P, KCH, CO], BF16, isOutput=False)
    wv_ext = nc.declare_dram_parameter("wvT", [P, KCH, CO], BF16, isOutput=False)
    bias_ext = {}
    for nm in ("bq", "bk", "bqs", "bks", "bv"):
        bias_ext[nm] = nc.declare_dram_parameter(nm, [CO, 1], F32, isOutput=False)
    tab_ext = {}
    for nm in ("cosq", "sinq", "cosk", "sink"):
        tab_ext[nm] = nc.declare_dram_parameter(nm, [P, T], BF16, isOutput=False)
    out_ext = nc.declare_dram_parameter("out", [CO, T], F32, isOutput=True)

    ExpF = mybir.ActivationFunctionType.Exp
    Mul = mybir.AluOpType.mult
    Add = mybir.AluOpType.add

    with tile.TileContext(nc) as tc:
        from contextlib import ExitStack

        stack_all = ExitStack()
        consts = stack_all.enter_context(tc.tile_pool(name="consts", bufs=1))
        persist = stack_all.enter_context(tc.tile_pool(name="persist", bufs=1))
        xw = stack_all.enter_context(tc.tile_pool(name="xw", bufs=1))
        projtmp = stack_all.enter_context(tc.tile_pool(name="projtmp", bufs=2))
        scp = stack_all.enter_context(
            tc.tile_pool(name="scp", bufs=2, space="PSUM")
        )
        expp = stack_all.enter_context(tc.tile_pool(name="expp", bufs=31))
        outp = stack_all.enter_context(tc.tile_pool(name="outp", bufs=2))
        smallp = stack_all.enter_context(tc.tile_pool(name="small", bufs=1))

        # ---------------- input DMAs: few, large, 128-partition ----------
        # SP ring: xq h0, xkv h0, xq h1, xkv h1 (2MB each).
        # Act ring: wq, cos/sin(q), wk, wv, cos/sin(k).
        # GpSimd ring: bias columns.
        x_sb = {}
        for name in ("q", "kv"):
            for h in range(XH):
                x_sb[(name, h)] = xw.tile([P, KCH, TH], BF16, tag=f"x{name}{h}",
                                          name=f"x{name}{h}")

        def load_x(name, exts, h):
            nc.sync.dma_start(
                out=x_sb[(name, h)][:],
                in_=bass.AP(
                    tensor=exts[h], offset=0,
                    ap=[[KCH * TH, P], [1, KCH * TH]],
                ),
            )

        w_sb = {}
        tabs = {}

        def load_w(name, ext):
            wb = xw.tile([P, KCH, CO], BF16, tag=f"w{name}", name=f"w{name}")
            nc.sync.dma_start(
                out=wb[:],
                in_=bass.AP(tensor=ext, offset=0,
                            ap=[[KCH * CO, P], [1, KCH * CO]]),
            )
            w_sb["w" + name] = wb

        def load_tab(nm):
            t_ = consts.tile([P, T], BF16, tag=nm)
            nc.scalar.dma_start(out=t_[:], in_=tab_ext[nm][:, :])
            tabs[nm] = t_

        # SP ring (deps first): wq, wk, xq h0, xkv h0, wv, xq h1, xkv h1.
        # Act ring: rotary tables.
        load_w("q", wq_ext)
        load_w("k", wk_ext)
        load_x("q", xq_ext, 0)
        load_x("kv", xkv_ext, 0)
        load_w("v", wv_ext)
        load_x("q", xq_ext, 1)
        load_x("kv", xkv_ext, 1)
        load_tab("cosq")
        load_tab("sinq")
        load_tab("cosk")
        load_tab("sink")

        bias_cols = {}
        for nm in ("bq", "bk", "bqs", "bks"):
            for ct in range(NCT):
                t_ = consts.tile([P, 1], F32, tag=f"{nm}{ct}")
                nc.gpsimd.dma_start(
                    out=t_[:], in_=bias_ext[nm][ct * P : (ct + 1) * P, :]
                )
                bias_cols[(nm, ct)] = t_
        bvb_sb = consts.tile([P, CO], F32, tag="bvb")
        nc.gpsimd.dma_start(
            out=bvb_sb[:],
            in_=bass.AP(tensor=bias_ext["bv"], offset=0, ap=[[0, P], [1, CO]]),
        )
        # persistent rotated q/k and v tiles
        rot_sb = {}
        for name in ("q", "k"):
            for ct in range(NCT):
                rot_sb[(name, ct)] = persist.tile(
                    [P, T], BF16, tag=f"r{name}{ct}", name=f"r{name}{ct}"
                )
        v_sb = [
            persist.tile([P, HEADS_PER_CORE, HEAD_DIM + 1], BF16,
                         tag=f"v{st}", name=f"v{st}")
            for st in range(NST)
        ]

        stack_p = ExitStack()
        projp = stack_p.enter_context(
            tc.tile_pool(name="projp", bufs=2, space="PSUM")
        )

        # ---------------- q/k projection + rotary (one t-half) ----------
        def proj_group(name, xsrc, ct, half):
            dst = rot_sb[(name, ct)]
            cos_t = tabs["cos" + name]
            sin_t = tabs["sin" + name]
            ps = projp.tile([P, TH], F32, tag="pj",
                            name=f"pj{name}{ct}_{half}")
            for k in range(KCH):
                for i in range(2):
                    nc.tensor.matmul(
                        ps[:, i * TCH : (i + 1) * TCH],
                        w_sb["w" + name][:, k, ct * P : (ct + 1) * P],
                        x_sb[(xsrc, half)][:, k, i * TCH : (i + 1) * TCH],
                        start=(k == 0),
                        stop=(k == KCH - 1),
                    )
            for i in range(2):
                tsl = slice(half * TH + i * TCH, half * TH + (i + 1) * TCH)
                psl = slice(i * TCH, (i + 1) * TCH)
                # rq = (q+b)*cos + (swap(q)+swap(b))*sin_signed
                qsb = projtmp.tile([P, TCH], BF16, tag="qsb",
                                   name=f"qsb{name}{ct}{half}{i}")
                nc.vector.tensor_scalar_add(
                    qsb[:], ps[:, psl], bias_cols[("b" + name, ct)][:]
                )
                qsw = projtmp.tile([P, TCH], F32, tag="qsw",
                                   name=f"qsw{name}{ct}{half}{i}")
                nc.vector.stream_shuffle(qsw[:], ps[:, psl], _SWAP_MASK)
                t2 = projtmp.tile([P, TCH], BF16, tag="rot2",
                                  name=f"t2{name}{ct}{half}{i}")
                nc.vector.scalar_tensor_tensor(
                    t2[:], qsw[:], bias_cols[("b" + name + "s", ct)][:],
                    sin_t[:, tsl], op0=Add, op1=Mul,
                )
                nc.vector.tensor_mul(dst[:, tsl], qsb[:], cos_t[:, tsl])
                nc.vector.tensor_add(dst[:, tsl], dst[:, tsl], t2[:])

        # pair-0 projections (DMA-paced; emission order matches DMA
        # arrival so the in-order PE queue never head-of-line blocks)
        proj_group("q", "q", 0, 0)
        proj_group("k", "kv", 0, 0)
        proj_group("q", "q", 0, 1)
        proj_group("k", "kv", 0, 1)

        # v projection ([128,256] fits a projp slot); emitted after the
        # it0 scores so it backfills PE idle instead of preempting them
        def emit_v_proj():
            for st in range(NST):
                vt = v_sb[st]
                psv = projp.tile([P, CO], F32, tag="pj", name=f"psv{st}")
                half, col = divmod(st * P, TH)
                for k in range(KCH):
                    nc.tensor.matmul(
                        psv[:],
                        x_sb[("kv", half)][:, k, col : col + P],
                        w_sb["wv"][:, k, :],
                        start=(k == 0),
                        stop=(k == KCH - 1),
                    )
                nc.vector.tensor_add(
                    vt[:, :, 0:HEAD_DIM],
                    psv[:].rearrange("p (h d) -> p h d", h=HEADS_PER_CORE),
                    bvb_sb[:].rearrange("p (h d) -> p h d", h=HEADS_PER_CORE),
                )
                nc.vector.memset(vt[:, :, HEAD_DIM : HEAD_DIM + 1], 1.0)

        # ---------------- attention ----------------
        ITERS = [(p_, t_) for p_ in range(NCT) for t_ in range(2)]

        def scores_exp(it, st):
            pair, th = ITERS[it]
            rk = rot_sb[("k", pair)]
            rq = rot_sb[("q", pair)]
            pss = [
                scp.tile([P, TH], F32, tag="sc", name=f"sc{it}_{st}_{h}")
                for h in range(2)
            ]
            # wave order: per tcc, all 4 (h, sh) quadrant MMs back-to-back
            # (disjoint row+col groups, 2 streams) -> concurrent execution
            for tcc in range(2):
                tsl = slice(th * TH + tcc * TCH, th * TH + (tcc + 1) * TCH)
                psl = slice(tcc * TCH, (tcc + 1) * TCH)
                for h in range(2):
                    for sh in range(2):
                        nc.tensor.matmul(
                            pss[h][sh * 64 : (sh + 1) * 64, psl],
                            rk[h * 64 : (h + 1) * 64,
                               st * P + sh * 64 : st * P + (sh + 1) * 64],
                            rq[h * 64 : (h + 1) * 64, tsl],
                            start=True, stop=True,
                            tile_position=(h * 64, sh * 64),
                        )
            etiles = []
            for h in range(2):
                e = expp.tile([P, TH], BF16, tag="exp", name=f"e{it}_{st}_{h}")
                nc.scalar.activation(e[:], pss[h][:], ExpF, scale=SCALE)
                etiles.append(e)
            return etiles

        def av_mms(it, st, vps_tcc, tcc, etiles):
            pair, th = ITERS[it]
            psl = slice(tcc * TCH, (tcc + 1) * TCH)
            for sub in range(2):
                h = pair * 2 + sub
                nc.tensor.matmul(
                    vps_tcc[sub][:],
                    v_sb[st][:, h, :],
                    etiles[st][sub][:, psl],
                    start=(st == 0),
                    stop=(st == NST - 1),
                )

        def epilogue(it, vps_by_tcc):
            pair, th = ITERS[it]
            for sub in range(2):
                h = pair * 2 + sub
                vcp = outp.tile([HEAD_DIM + 1, TH], F32, tag="vcp",
                                name=f"vcp{it}_{sub}")
                for tcc in range(2):
                    nc.vector.tensor_copy(
                        vcp[:, tcc * TCH : (tcc + 1) * TCH],
                        vps_by_tcc[tcc][sub][:],
                    )
                dn = smallp.tile([1, TH], F32, tag="dn",
                                 name=f"dn{it}_{sub}")
                nc.sync.dma_start(
                    out=dn[:], in_=vcp[HEAD_DIM : HEAD_DIM + 1, :]
                )
                nc.vector.reciprocal_approx_fast(out=dn[:], in_=dn[:])
                recb = smallp.tile([1, TH], BF16, tag="recb",
                                   name=f"recb{it}_{sub}")
                nc.vector.tensor_copy(recb[:], dn[:])
                # broadcast 1->64 partitions on GpSimd (keeps PE queue free)
                rcb = smallp.tile([HEAD_DIM, TH], BF16, tag="rcb",
                                  name=f"rcb{it}_{sub}")
                for j in range(2):
                    nc.gpsimd.partition_broadcast(
                        rcb[:, j * TCH : (j + 1) * TCH],
                        recb[:, j * TCH : (j + 1) * TCH],
                        channels=HEAD_DIM,
                    )
                nc.vector.tensor_mul(
                    vcp[0:HEAD_DIM, :], vcp[0:HEAD_DIM, :], rcb[:]
                )
                nc.sync.dma_start(
                    out=out_ext[h * HEAD_DIM : (h + 1) * HEAD_DIM,
                                th * TH : (th + 1) * TH],
                    in_=vcp[0:HEAD_DIM, :],
                )

        # window it0: scores only (v/pair-1 projections backfill PE idle)
        et = {0: [scores_exp(0, st) for st in range(NST)]}

        emit_v_proj()

        # pair-1 projections (needed by it2; run in windows it0/it1)
        proj_group("q", "q", 1, 0)
        proj_group("k", "kv", 1, 0)
        proj_group("q", "q", 1, 1)
        proj_group("k", "kv", 1, 1)
        stack_p.close()

        psva = stack_all.enter_context(
            tc.tile_pool(name="psva", bufs=4, space="PSUM")
        )

        def new_vps(it, tcc):
            return [
                psva.tile([HEAD_DIM + 1, TCH], F32, tag="va",
                          name=f"vacc{it}_{tcc}_{s}")
                for s in range(2)
            ]

        # windows it1..it3: scores(it) + AV(it-1) tcc0 chain inline,
        # tcc1 chain backfills; AV(it3) chases exp(it3) at the end.
        vps = {}
        for it in range(1, 4):
            vps[(it - 1, 0)] = new_vps(it - 1, 0)
            vps[(it - 1, 1)] = new_vps(it - 1, 1)
            et[it] = []
            for st in range(NST):
                et[it].append(scores_exp(it, st))
                av_mms(it - 1, st, vps[(it - 1, 0)], 0, et[it - 1])
                av_mms(it - 1, st, vps[(it - 1, 1)], 1, et[it - 1])
            epilogue(it - 1, [vps[(it - 1, 0)], vps[(it - 1, 1)]])
        vps[(3, 0)] = new_vps(3, 0)
        vps[(3, 1)] = new_vps(3, 1)
        for st in range(NST):
            av_mms(3, st, vps[(3, 0)], 0, et[3])
            av_mms(3, st, vps[(3, 1)], 1, et[3])
        epilogue(3, [vps[(3, 0)], vps[(3, 1)]])

        stack_all.close()
    nc.finalize()
    return nc


def make_in_maps(x_q, x_kv, q_positions, kv_positions, Wq, bq, Wk, bk, Wv, bv):
    x_q = np.asarray(x_q, np.float32)
    x_kv = np.asarray(x_kv, np.float32)
    q_positions = np.asarray(q_positions, np.int32)
    kv_positions = np.asarray(kv_positions, np.int32)
    Wq, Wk, Wv = (np.asarray(w, np.float32) for w in (Wq, Wk, Wv))
    bq, bk, bv = (np.asarray(b, np.float32) for b in (bq, bk, bv))

    xqT = [np.ascontiguousarray(x_q[b_].T).astype(NPBF16) for b_ in range(B)]
    xkvT = [np.ascontiguousarray(x_kv[b_].T).astype(NPBF16) for b_ in range(B)]
    tabs = []
    for b_ in range(B):
        cq, sq = _rot_tables(q_positions[b_])
        ck, sk = _rot_tables(kv_positions[b_])
        tabs.append((cq, sq, ck, sk))

    # pair-swapped bias vectors for the rotary shuffle path (swap within
    # rotated channels of each 64-channel head slot; identity elsewhere --
    # non-rotated channels multiply a zero sin so identity is harmless)
    swap_idx = np.arange(C)
    r = swap_idx % HEAD_DIM < N_ROT
    swap_idx[r] = swap_idx[r] ^ 1
    bqs_full = bq[swap_idx]
    bks_full = bk[swap_idx]

    in_maps = []
    for core in range(N_CORES):
        b_, hg = divmod(core, N_CORES // B)
        hsl = slice(hg * CO, (hg + 1) * CO)
        cq, sq, ck, sk = tabs[b_]
        def prepack(wT):  # [C, n] -> [P, KCH, n] partition-major
            n = wT.shape[1]
            return np.ascontiguousarray(
                wT.reshape(KCH, P, n).transpose(1, 0, 2)
            )

        m = {
            "wqT": prepack(Wq[hsl].T.astype(NPBF16)),
            "wkT": prepack(Wk[hsl].T.astype(NPBF16)),
            "wvT": prepack(Wv[hsl].T.astype(NPBF16)),
            "bq": np.ascontiguousarray(bq[hsl][:, None]),
            "bk": np.ascontiguousarray(bk[hsl][:, None]),
            "bqs": np.ascontiguousarray(bqs_full[hsl][:, None]),
            "bks": np.ascontiguousarray(bks_full[hsl][:, None]),
            "bv": np.ascontiguousarray(bv[hsl][:, None]),
            "cosq": cq, "sinq": sq, "cosk": ck, "sink": sk,
        }
        for h in range(XH):
            m[f"xqT{h}"] = prepack(xqT[b_][:, h * TH : (h + 1) * TH])
            m[f"xkvT{h}"] = prepack(xkvT[b_][:, h * TH : (h + 1) * TH])
        in_maps.append(m)
    return in_maps


_CACHED = {}


def kernel(x_q, x_kv, q_positions, kv_positions, Wq, bq, Wk, bk, Wv, bv):
    in_maps = make_in_maps(
        x_q, x_kv, q_positions, kv_positions, Wq, bq, Wk, bk, Wv, bv
    )
    if "nc" not in _CACHED:
        _CACHED["nc"] = build_bass()
    nc = _CACHED["nc"]

    res = run_bass_kernel_spmd(nc, in_maps, core_ids=list(range(N_CORES)))
    out = np.empty((B, T, C), np.float32)
    for core in range(N_CORES):
        b_, hg = divmod(core, N_CORES // B)
        out[b_, :, hg * CO : (hg + 1) * CO] = res.results[core]["out"].T
    return out


# revision 26
# speedup vs baseline: 1.0741x; 1.0741x over previous
"""
Bass/Trainium2 kernel for nn_Attention_72456098284196.

Attention module: QKV projections + partial rotary (first 32 of 64 head
channels, all heads) + softmax attention.  B=2, T=2048, C=1024, H=16, D=64.

Sharding: 8 NeuronCores = 2 batches x 4 head-groups (4 heads each).
Pure tensor/batch parallel -> no collectives; host slices inputs and
concatenates outputs.

Host prep: x / W are cast to bf16 and transposed; the rotary cos/sin
tables are precomputed from the position inputs ([128, T] bf16, one row
per channel of a 2-head c-tile; the splice sign is folded into the sin
table so the device-side splice is a plain partition pair-swap).

Device-side dataflow (per core, matmuls bf16 with fp32 PSUM accum):
  DMA: few large 128-partition transfers (x as two 2MB t-halves per
       tensor) so pair-0 projections start ~6us in.
  qT[c,t] = WqT.T @ xqT   (c = 4 heads x 64 ch, two 128-partition c-tiles)
  rotary:  rq = (q+b)*cos + (shuffle(q)+swap(b))*sin_signed on DVE
           (stream_shuffle pair-swap; passthrough channels cos=1,sin=0)
  scores^T[s,t] = rkT.T @ rqT per head, 64x64 PE tiling (two s-halves to
           disjoint col groups stream concurrently)
  expT = Exp(scores^T / 8) on ScalarE -> bf16.  ScalarE is the pacing
           engine (~1.14us per [128,1024] tile, 128 tiles = ~145us); the
           schedule keeps it fed from ~15us to the end:
             window it0: scores(it0) + q0/k0/v/q1/k1 projections backfill
             window itN: scores(itN) + AV(itN-1) backfill
  outT[d,t] accum over s of [v | 1].T @ expT  (M=65: row 64 accumulates
           the softmax denominator for free)
  normalize: recip(denominator row) via fast approx, broadcast 1->64
           partitions via a bf16 PE outer product, multiply, DMA out.
"""

import math
import sys

import numpy as np

if "/opt/trn_rl_repo" not in sys.path:
    sys.path.insert(0, "/opt/trn_rl_repo")

import concourse.bass as bass  # noqa: E402
import concourse.mybir as mybir  # noqa: E402
import concourse.tile as tile  # noqa: E402
from concourse import bacc  # noqa: E402
from concourse.bass_utils import run_bass_kernel_spmd  # noqa: E402

B, T, C = 2, 2048, 1024
NUM_HEADS = 16
HEAD_DIM = 64
N_CORES = 8
HEADS_PER_CORE = NUM_HEADS // (N_CORES // B)  # 4
CO = HEADS_PER_CORE * HEAD_DIM  # 256 out channels per core
N_ROT = 32  # rotated channels per head
MAX_WAVELENGTH = 8192.0

F32 = mybir.dt.float32
BF16 = mybir.dt.bfloat16
NPBF16 = mybir.dt.np(BF16)

P = 128  # partitions
TCH = 512  # matmul N chunk (1 PSUM bank)
KCH = C // P  # 8 contraction chunks
NCT = CO // P  # 2 c-tiles (each = 2 heads x 64)
NST = T // P  # 16 s tiles
SCALE = 1.0 / math.sqrt(HEAD_DIM)
TH = 1024  # attention t-half width / x DMA half width
XH = 2


def _inv_freq() -> np.ndarray:
    """[32] inverse frequencies (pairs repeated), matching the reference."""
    num_bands = N_ROT // 2  # 16
    freq = MAX_WAVELENGTH ** (
        2.0 / N_ROT * np.linspace(0.0, num_bands, num_bands, dtype=np.float64)
    )
    return np.repeat(1.0 / freq, 2)  # [32]


def _rot_tables(pos: np.ndarray) -> tuple[np.ndarray, np.ndarray]:
    """cos/sin tables [128, T] bf16 for a 2-head c-tile.

    Rows r in [0,32) and [64,96): rotary channels (cos/sin of pos*invf);
    other rows: cos=1, sin=0 (passthrough).  The splice negation is
    folded into sin: even channels get -sin so that
    rq = q*cos + pairswap(q)*sin_signed.
    """
    inv = _inv_freq()  # [32]
    rad = pos.astype(np.float64)[None, :] * inv[:, None]  # [32, T]
    cos32 = np.cos(rad)
    sin32 = np.sin(rad)
    sign = np.where(np.arange(N_ROT) % 2 == 0, -1.0, 1.0)[:, None]
    sin32 = sin32 * sign
    cos = np.zeros((P, pos.shape[0]), np.float64)
    sin = np.zeros((P, pos.shape[0]), np.float64)
    for o in (0, 64):
        cos[o : o + N_ROT] = cos32
        sin[o : o + N_ROT] = sin32
        cos[o + N_ROT : o + 64] = 1.0
    return cos.astype(NPBF16), sin.astype(NPBF16)


_SWAP_MASK = [i ^ 1 for i in range(32)]  # pair swap within each 32-quadrant


def build_bass() -> bass.Bass:
    nc = bacc.Bacc()

    # x / w are host-prepacked partition-major ([P, KCH, ...]) so each
    # DMA moves 128 x 16KB contiguous per-partition runs.
    xq_ext = [
        nc.declare_dram_parameter(f"xqT{h}", [P, KCH, TH], BF16, isOutput=False)
        for h in range(XH)
    ]
    xkv_ext = [
        nc.declare_dram_parameter(f"xkvT{h}", [P, KCH, TH], BF16, isOutput=False)
        for h in range(XH)
    ]
    wq_ext = nc.declare_dram_parameter("wqT", [P, KCH, CO], BF16, isOutput=False)
    wk_ext = nc.declare_dram_parameter("wkT", [P, KCH, CO], BF16, isOutput=False)
    wv_ext = nc.declare_dram_parameter("wvT", [P, KCH, CO], BF16, isOutput=False)
    bias_ext = {}
    for nm in ("bq", "bk", "bqs", "bks", "bv"):
        bias_ext[nm] = nc.declare_dram_parameter(nm, [CO, 1], F32, isOutput=False)
    tab_ext = {}
    for nm in ("cosq", "sinq", "cosk", "sink"):
        tab_ext[nm] = nc.declare_dram_parameter(nm, [P, T], BF16, isOutput=False)
    out_ext = nc.declare_dram_parameter("out", [CO, T], F32, isOutput=True)

    ExpF = mybir.ActivationFunctionType.Exp
    Mul = mybir.AluOpType.mult
    Add = mybir.AluOpType.add

    with tile.TileContext(nc) as tc:
        from contextlib import ExitStack

        stack_all = ExitStack()
        consts = stack_all.enter_context(tc.tile_pool(name="consts", bufs=1))
        persist = stack_all.enter_context(tc.tile_pool(name="persist", bufs=1))
        xw = stack_all.enter_context(tc.tile_pool(name="xw", bufs=1))
        projtmp = stack_all.enter_context(tc.tile_pool(name="projtmp", bufs=2))
        scp = stack_all.enter_context(
            tc.tile_pool(name="scp", bufs=2, space="PSUM")
        )
        expp = stack_all.enter_context(tc.tile_pool(name="expp", bufs=31))
        outp = stack_all.enter_context(tc.tile_pool(name="outp", bufs=2))
        smallp = stack_all.enter_context(tc.tile_pool(name="small", bufs=1))

        # ---------------- input DMAs: few, large, 128-partition ----------
        # SP ring: xq h0, xkv h0, xq h1, xkv h1 (2MB each).
        # Act ring: wq, cos/sin(q), wk, wv, cos/sin(k).
        # GpSimd ring: bias columns.
        x_sb = {}
        for name in ("q", "kv"):
            for h in range(XH):
                x_sb[(name, h)] = xw.tile([P, KCH, TH], BF16, tag=f"x{name}{h}",
                                          name=f"x{name}{h}")

        def load_x(name, exts, h):
            nc.sync.dma_start(
                out=x_sb[(name, h)][:],
                in_=bass.AP(
                    tensor=exts[h], offset=0,
                    ap=[[KCH * TH, P], [1, KCH * TH]],
                ),
            )

        w_sb = {}
        tabs = {}

        def load_w(name, ext):
            wb = xw.tile([P, KCH, CO], BF16, tag=f"w{name}", name=f"w{name}")
            nc.scalar.dma_start(
                out=wb[:],
                in_=bass.AP(tensor=ext, offset=0,
                            ap=[[KCH * CO, P], [1, KCH * CO]]),
            )
            w_sb["w" + name] = wb

        def load_tab(nm):
            t_ = consts.tile([P, T], BF16, tag=nm)
            nc.scalar.dma_start(out=t_[:], in_=tab_ext[nm][:, :])
            tabs[nm] = t_

        # Act ring: weights + tables; SP ring: x.  The startup-critical
        # set (wq, wk, xq h0, xkv h0) is split across both rings.
        load_w("q", wq_ext)
        load_w("k", wk_ext)
        load_x("q", xq_ext, 0)
        load_x("kv", xkv_ext, 0)
        load_tab("cosq")
        load_tab("sinq")
        load_w("v", wv_ext)
        load_x("q", xq_ext, 1)
        load_x("kv", xkv_ext, 1)
        load_tab("cosk")
        load_tab("sink")

        bias_cols = {}
        for nm in ("bq", "bk", "bqs", "bks"):
            for ct in range(NCT):
                t_ = consts.tile([P, 1], F32, tag=f"{nm}{ct}")
                nc.gpsimd.dma_start(
                    out=t_[:], in_=bias_ext[nm][ct * P : (ct + 1) * P, :]
                )
                bias_cols[(nm, ct)] = t_
        bvb_sb = consts.tile([P, CO], F32, tag="bvb")
        nc.gpsimd.dma_start(
            out=bvb_sb[:],
            in_=bass.AP(tensor=bias_ext["bv"], offset=0, ap=[[0, P], [1, CO]]),
        )
        # persistent rotated q/k and v tiles
        rot_sb = {}
        for name in ("q", "k"):
            for ct in range(NCT):
                rot_sb[(name, ct)] = persist.tile(
                    [P, T], BF16, tag=f"r{name}{ct}", name=f"r{name}{ct}"
                )
        v_sb = [
            persist.tile([P, HEADS_PER_CORE, HEAD_DIM + 1], BF16,
                         tag=f"v{st}", name=f"v{st}")
            for st in range(NST)
        ]

        stack_p = ExitStack()
        projp = stack_p.enter_context(
            tc.tile_pool(name="projp", bufs=2, space="PSUM")
        )

        # ---------------- q/k projection + rotary (one t-half) ----------
        def proj_group(name, xsrc, ct, half):
            dst = rot_sb[(name, ct)]
            cos_t = tabs["cos" + name]
            sin_t = tabs["sin" + name]
            ps = projp.tile([P, TH], F32, tag="pj",
                            name=f"pj{name}{ct}_{half}")
            for k in range(KCH):
                for i in range(2):
                    nc.tensor.matmul(
                        ps[:, i * TCH : (i + 1) * TCH],
                        w_sb["w" + name][:, k, ct * P : (ct + 1) * P],
                        x_sb[(xsrc, half)][:, k, i * TCH : (i + 1) * TCH],
                        start=(k == 0),
                        stop=(k == KCH - 1),
                    )
            for i in range(2):
                tsl = slice(half * TH + i * TCH, half * TH + (i + 1) * TCH)
                psl = slice(i * TCH, (i + 1) * TCH)
                # rq = (q+b)*cos + (swap(q)+swap(b))*sin_signed
                qsb = projtmp.tile([P, TCH], BF16, tag="qsb",
                                   name=f"qsb{name}{ct}{half}{i}")
                nc.vector.tensor_scalar_add(
                    qsb[:], ps[:, psl], bias_cols[("b" + name, ct)][:]
                )
                qsw = projtmp.tile([P, TCH], F32, tag="qsw",
                                   name=f"qsw{name}{ct}{half}{i}")
                nc.vector.stream_shuffle(qsw[:], ps[:, psl], _SWAP_MASK)
                t2 = projtmp.tile([P, TCH], BF16, tag="rot2",
                                  name=f"t2{name}{ct}{half}{i}")
                nc.vector.scalar_tensor_tensor(
                    t2[:], qsw[:], bias_cols[("b" + name + "s", ct)][:],
                    sin_t[:, tsl], op0=Add, op1=Mul,
                )
                nc.vector.tensor_mul(dst[:, tsl], qsb[:], cos_t[:, tsl])
                nc.vector.tensor_add(dst[:, tsl], dst[:, tsl], t2[:])

        # pair-0 projections (DMA-paced; emission order matches DMA
        # arrival so the in-order PE queue never head-of-line blocks)
        proj_group("q", "q", 0, 0)
        proj_group("k", "kv", 0, 0)
        proj_group("q", "q", 0, 1)
        proj_group("k", "kv", 0, 1)

        # v projection ([128,256] fits a projp slot); emitted after the
        # it0 scores so it backfills PE idle instead of preempting them
        def emit_v_proj():
            for st in range(NST):
                vt = v_sb[st]
                psv = projp.tile([P, CO], F32, tag="pj", name=f"psv{st}")
                half, col = divmod(st * P, TH)
                for k in range(KCH):
                    nc.tensor.matmul(
                        psv[:],
                        x_sb[("kv", half)][:, k, col : col + P],
                        w_sb["wv"][:, k, :],
                        start=(k == 0),
                        stop=(k == KCH - 1),
                    )
                nc.vector.tensor_add(
                    vt[:, :, 0:HEAD_DIM],
                    psv[:].rearrange("p (h d) -> p h d", h=HEADS_PER_CORE),
                    bvb_sb[:].rearrange("p (h d) -> p h d", h=HEADS_PER_CORE),
                )
                nc.vector.memset(vt[:, :, HEAD_DIM : HEAD_DIM + 1], 1.0)

        # ---------------- attention ----------------
        ITERS = [(p_, t_) for p_ in range(NCT) for t_ in range(2)]

        def scores_exp(it, st):
            pair, th = ITERS[it]
            rk = rot_sb[("k", pair)]
            rq = rot_sb[("q", pair)]
            pss = [
                scp.tile([P, TH], F32, tag="sc", name=f"sc{it}_{st}_{h}")
                for h in range(2)
            ]
            # wave order: per tcc, all 4 (h, sh) quadrant MMs back-to-back
            # (disjoint row+col groups, 2 streams) -> concurrent execution
            for tcc in range(2):
                tsl = slice(th * TH + tcc * TCH, th * TH + (tcc + 1) * TCH)
                psl = slice(tcc * TCH, (tcc + 1) * TCH)
                for h in range(2):
                    for sh in range(2):
                        nc.tensor.matmul(
                            pss[h][sh * 64 : (sh + 1) * 64, psl],
                            rk[h * 64 : (h + 1) * 64,
                               st * P + sh * 64 : st * P + (sh + 1) * 64],
                            rq[h * 64 : (h + 1) * 64, tsl],
                            start=True, stop=True,
                            tile_position=(h * 64, sh * 64),
                        )
            etiles = []
            for h in range(2):
                e = expp.tile([P, TH], BF16, tag="exp", name=f"e{it}_{st}_{h}")
                nc.scalar.activation(e[:], pss[h][:], ExpF, scale=SCALE)
                etiles.append(e)
            return etiles

        def av_mms(it, st, vps, etiles):
            pair, th = ITERS[it]
            for sub in range(2):
                h = pair * 2 + sub
                e = etiles[st][sub]
                for tcc in range(2):
                    psl = slice(tcc * TCH, (tcc + 1) * TCH)
                    nc.tensor.matmul(
                        vps[sub][:, psl],
                        v_sb[st][:, h, :],
                        e[:, psl],
                        start=(st == 0),
                        stop=(st == NST - 1),
                    )

        def epilogue(it, vps):
            pair, th = ITERS[it]
            for sub in range(2):
                h = pair * 2 + sub
                vcp = outp.tile([HEAD_DIM + 1, TH], F32, tag="vcp",
                                name=f"vcp{it}_{sub}")
                nc.vector.tensor_copy(vcp[:], vps[sub][:])
                dn = smallp.tile([1, TH], F32, tag="dn",
                                 name=f"dn{it}_{sub}")
                nc.sync.dma_start(
                    out=dn[:], in_=vcp[HEAD_DIM : HEAD_DIM + 1, :]
                )
                nc.vector.reciprocal_approx_fast(out=dn[:], in_=dn[:])
                recb = smallp.tile([1, TH], BF16, tag="recb",
                                   name=f"recb{it}_{sub}")
                nc.vector.tensor_copy(recb[:], dn[:])
                # broadcast 1->64 partitions on GpSimd (keeps PE queue free)
                rcb = smallp.tile([HEAD_DIM, TH], BF16, tag="rcb",
                                  name=f"rcb{it}_{sub}")
                for j in range(2):
                    nc.gpsimd.partition_broadcast(
                        rcb[:, j * TCH : (j + 1) * TCH],
                        recb[:, j * TCH : (j + 1) * TCH],
                        channels=HEAD_DIM,
                    )
                nc.vector.tensor_mul(
                    vcp[0:HEAD_DIM, :], vcp[0:HEAD_DIM, :], rcb[:]
                )
                nc.sync.dma_start(
                    out=out_ext[h * HEAD_DIM : (h + 1) * HEAD_DIM,
                                th * TH : (th + 1) * TH],
                    in_=vcp[0:HEAD_DIM, :],
                )

        emit_v_proj()

        # window it0: scores only (projections backfill PE idle)
        et = {0: [scores_exp(0, st) for st in range(NST)]}

        # pair-1 projections (needed by it2; run in windows it0/it1)
        proj_group("q", "q", 1, 0)
        proj_group("k", "kv", 1, 0)
        proj_group("q", "q", 1, 1)
        proj_group("k", "kv", 1, 1)
        stack_p.close()

        psva = stack_all.enter_context(
            tc.tile_pool(name="psva", bufs=2, space="PSUM")
        )

        def new_vps(it):
            return [
                psva.tile([HEAD_DIM + 1, TH], F32, tag="va",
                          name=f"vacc{it}_{s}")
                for s in range(2)
            ]

        # windows it1..it3: scores(it) + AV(it-1); then AV(it3) + epilogues
        vps = {}
        for it in range(1, 4):
            vps[it - 1] = new_vps(it - 1)
            et[it] = []
            for st in range(NST):
                et[it].append(scores_exp(it, st))
                av_mms(it - 1, st, vps[it - 1], et[it - 1])
            epilogue(it - 1, vps[it - 1])
        vps[3] = new_vps(3)
        for st in range(NST):
            av_mms(3, st, vps[3], et[3])
        epilogue(3, vps[3])

        stack_all.close()
    nc.finalize()
    return nc


def make_in_maps(x_q, x_kv, q_positions, kv_positions, Wq, bq, Wk, bk, Wv, bv):
    x_q = np.asarray(x_q, np.float32)
    x_kv = np.asarray(x_kv, np.float32)
    q_positions = np.asarray(q_positions, np.int32)
    kv_positions = np.asarray(kv_positions, np.int32)
    Wq, Wk, Wv = (np.asarray(w, np.float32) for w in (Wq, Wk, Wv))
    bq, bk, bv = (np.asarray(b, np.float32) for b in (bq, bk, bv))

    xqT = [np.ascontiguousarray(x_q[b_].T).astype(NPBF16) for b_ in range(B)]
    xkvT = [np.ascontiguousarray(x_kv[b_].T).astype(NPBF16) for b_ in range(B)]
    tabs = []
    for b_ in range(B):
        cq, sq = _rot_tables(q_positions[b_])
        ck, sk = _rot_tables(kv_positions[b_])
        tabs.append((cq, sq, ck, sk))

    # pair-swapped bias vectors for the rotary shuffle path (swap within
    # rotated channels of each 64-channel head slot; identity elsewhere --
    # non-rotated channels multiply a zero sin so identity is harmless)
    swap_idx = np.arange(C)
    r = swap_idx % HEAD_DIM < N_ROT
    swap_idx[r] = swap_idx[r] ^ 1
    bqs_full = bq[swap_idx]
    bks_full = bk[swap_idx]

    in_maps = []
    for core in range(N_CORES):
        b_, hg = divmod(core, N_CORES // B)
        hsl = slice(hg * CO, (hg + 1) * CO)
        cq, sq, ck, sk = tabs[b_]
        def prepack(wT):  # [C, n] -> [P, KCH, n] partition-major
            n = wT.shape[1]
            return np.ascontiguousarray(
                wT.reshape(KCH, P, n).transpose(1, 0, 2)
            )

        m = {
            "wqT": prepack(Wq[hsl].T.astype(NPBF16)),
            "wkT": prepack(Wk[hsl].T.astype(NPBF16)),
            "wvT": prepack(Wv[hsl].T.astype(NPBF16)),
            "bq": np.ascontiguousarray(bq[hsl][:, None]),
            "bk": np.ascontiguousarray(bk[hsl][:, None]),
            "bqs": np.ascontiguousarray(bqs_full[hsl][:, None]),
            "bks": np.ascontiguousarray(bks_full[hsl][:, None]),
            "bv": np.ascontiguousarray(bv[hsl][:, None]),
            "cosq": cq, "sinq": sq, "cosk": ck, "sink": sk,
        }
        for h in range(XH):
            m[f"xqT{h}"] = prepack(xqT[b_][:, h * TH : (h + 1) * TH])
            m[f"xkvT{h}"] = prepack(xkvT[b_][:, h * TH : (h + 1) * TH])
        in_maps.append(m)
    return in_maps


_CACHED = {}


def kernel(x_q, x_kv, q_positions, kv_positions, Wq, bq, Wk, bk, Wv, bv):
    in_maps = make_in_maps(
        x_q, x_kv, q_positions, kv_positions, Wq, bq, Wk, bk, Wv, bv
    )
    if "nc" not in _CACHED:
        _CACHED["nc"] = build_bass()
    nc = _CACHED["nc"]

    res = run_bass_kernel_spmd(nc, in_maps, core_ids=list(range(N_CORES)))
    out = np.empty((B, T, C), np.float32)
    for core in range(N_CORES):
        b_, hg = divmod(core, N_CORES // B)
        out[b_, :, hg * CO : (hg + 1) * CO] = res.results[core]["out"].T
    return out


# revision 27
# speedup vs baseline: 1.0804x; 1.0058x over previous
"""
Bass/Trainium2 kernel for nn_Attention_72456098284196.

Attention module: QKV projections + partial rotary (first 32 of 64 head
channels, all heads) + softmax attention.  B=2, T=2048, C=1024, H=16, D=64.

Sharding: 8 NeuronCores = 2 batches x 4 head-groups (4 heads each).
Pure tensor/batch parallel -> no collectives; host slices inputs and
concatenates outputs.

Host prep: x / W are cast to bf16 and transposed; the rotary cos/sin
tables are precomputed from the position inputs ([128, T] bf16, one row
per channel of a 2-head c-tile; the splice sign is folded into the sin
table so the device-side splice is a plain partition pair-swap).

Device-side dataflow (per core, matmuls bf16 with fp32 PSUM accum):
  DMA: few large 128-partition transfers (x as two 2MB t-halves per
       tensor) so pair-0 projections start ~6us in.
  qT[c,t] = WqT.T @ xqT   (c = 4 heads x 64 ch, two 128-partition c-tiles)
  rotary:  rq = (q+b)*cos + (shuffle(q)+swap(b))*sin_signed on DVE
           (stream_shuffle pair-swap; passthrough channels cos=1,sin=0)
  scores^T[s,t] = rkT.T @ rqT per head, 64x64 PE tiling (two s-halves to
           disjoint col groups stream concurrently)
  expT = Exp(scores^T / 8) on ScalarE -> bf16.  ScalarE is the pacing
           engine (~1.14us per [128,1024] tile, 128 tiles = ~145us); the
           schedule keeps it fed from ~15us to the end:
             window it0: scores(it0) + q0/k0/v/q1/k1 projections backfill
             window itN: scores(itN) + AV(itN-1) backfill
  outT[d,t] accum over s of [v | 1].T @ expT  (M=65: row 64 accumulates
           the softmax denominator for free)
  normalize: recip(denominator row) via fast approx, broadcast 1->64
           partitions via a bf16 PE outer product, multiply, DMA out.
"""

import math
import sys

import numpy as np

if "/opt/trn_rl_repo" not in sys.path:
    sys.path.insert(0, "/opt/trn_rl_repo")

import concourse.bass as bass  # noqa: E402
import concourse.mybir as mybir  # noqa: E402
import concourse.tile as tile  # noqa: E402
from concourse import bacc  # noqa: E402
from concourse.bass_utils import run_bass_kernel_spmd  # noqa: E402

B, T, C = 2, 2048, 1024
NUM_HEADS = 16
HEAD_DIM = 64
N_CORES = 8
HEADS_PER_CORE = NUM_HEADS // (N_CORES // B)  # 4
CO = HEADS_PER_CORE * HEAD_DIM  # 256 out channels per core
N_ROT = 32  # rotated channels per head
MAX_WAVELENGTH = 8192.0

F32 = mybir.dt.float32
BF16 = mybir.dt.bfloat16
NPBF16 = mybir.dt.np(BF16)

P = 128  # partitions
TCH = 512  # matmul N chunk (1 PSUM bank)
KCH = C // P  # 8 contraction chunks
NCT = CO // P  # 2 c-tiles (each = 2 heads x 64)
NST = T // P  # 16 s tiles
SCALE = 1.0 / math.sqrt(HEAD_DIM)
TH = 1024  # attention t-half width / x DMA half width
XH = 2


def _inv_freq() -> np.ndarray:
    """[32] inverse frequencies (pairs repeated), matching the reference."""
    num_bands = N_ROT // 2  # 16
    freq = MAX_WAVELENGTH ** (
        2.0 / N_ROT * np.linspace(0.0, num_bands, num_bands, dtype=np.float64)
    )
    return np.repeat(1.0 / freq, 2)  # [32]


def _rot_tables(pos: np.ndarray) -> tuple[np.ndarray, np.ndarray]:
    """cos/sin tables [128, T] bf16 for a 2-head c-tile.

    Rows r in [0,32) and [64,96): rotary channels (cos/sin of pos*invf);
    other rows: cos=1, sin=0 (passthrough).  The splice negation is
    folded into sin: even channels get -sin so that
    rq = q*cos + pairswap(q)*sin_signed.
    """
    inv = _inv_freq()  # [32]
    rad = pos.astype(np.float64)[None, :] * inv[:, None]  # [32, T]
    cos32 = np.cos(rad)
    sin32 = np.sin(rad)
    sign = np.where(np.arange(N_ROT) % 2 == 0, -1.0, 1.0)[:, None]
    sin32 = sin32 * sign
    cos = np.zeros((P, pos.shape[0]), np.float64)
    sin = np.zeros((P, pos.shape[0]), np.float64)
    for o in (0, 64):
        cos[o : o + N_ROT] = cos32
        sin[o : o + N_ROT] = sin32
        cos[o + N_ROT : o + 64] = 1.0
    return cos.astype(NPBF16), sin.astype(NPBF16)


_SWAP_MASK = [i ^ 1 for i in range(32)]  # pair swap within each 32-quadrant


def build_bass() -> bass.Bass:
    nc = bacc.Bacc()

    # x / w are host-prepacked partition-major ([P, KCH, ...]) so each
    # DMA moves 128 x 16KB contiguous per-partition runs.
    xq_ext = [
        nc.declare_dram_parameter(f"xqT{h}", [P, KCH, TH], BF16, isOutput=False)
        for h in range(XH)
    ]
    xkv_ext = [
        nc.declare_dram_parameter(f"xkvT{h}", [P, KCH, TH], BF16, isOutput=False)
        for h in range(XH)
    ]
    wq_ext = nc.declare_dram_parameter("wqT", [P, KCH, CO], BF16, isOutput=False)
    wk_ext = nc.declare_dram_parameter("wkT", [P, KCH, CO], BF16, isOutput=False)
    wv_ext = nc.declare_dram_parameter("wvT", [P, KCH, CO], BF16, isOutput=False)
    bias_ext = {}
    for nm in ("bq", "bk", "bqs", "bks", "bv"):
        bias_ext[nm] = nc.declare_dram_parameter(nm, [CO, 1], F32, isOutput=False)
    tab_ext = {}
    for nm in ("cosq", "sinq", "cosk", "sink"):
        tab_ext[nm] = nc.declare_dram_parameter(nm, [P, T], BF16, isOutput=False)
    out_ext = nc.declare_dram_parameter("out", [CO, T], F32, isOutput=True)

    ExpF = mybir.ActivationFunctionType.Exp
    Mul = mybir.AluOpType.mult
    Add = mybir.AluOpType.add

    with tile.TileContext(nc) as tc:
        from contextlib import ExitStack

        stack_all = ExitStack()
        consts = stack_all.enter_context(tc.tile_pool(name="consts", bufs=1))
        persist = stack_all.enter_context(tc.tile_pool(name="persist", bufs=1))
        xw = stack_all.enter_context(tc.tile_pool(name="xw", bufs=1))
        projtmp = stack_all.enter_context(tc.tile_pool(name="projtmp", bufs=2))
        scp = stack_all.enter_context(
            tc.tile_pool(name="scp", bufs=2, space="PSUM")
        )
        expp = stack_all.enter_context(tc.tile_pool(name="expp", bufs=31))
        outp = stack_all.enter_context(tc.tile_pool(name="outp", bufs=2))
        smallp = stack_all.enter_context(tc.tile_pool(name="small", bufs=1))

        # ---------------- input DMAs: few, large, 128-partition ----------
        # SP ring: xq h0, xkv h0, xq h1, xkv h1 (2MB each).
        # Act ring: wq, cos/sin(q), wk, wv, cos/sin(k).
        # GpSimd ring: bias columns.
        x_sb = {}
        for name in ("q", "kv"):
            for h in range(XH):
                x_sb[(name, h)] = xw.tile([P, KCH, TH], BF16, tag=f"x{name}{h}",
                                          name=f"x{name}{h}")

        def load_x(name, exts, h):
            nc.sync.dma_start(
                out=x_sb[(name, h)][:],
                in_=bass.AP(
                    tensor=exts[h], offset=0,
                    ap=[[KCH * TH, P], [1, KCH * TH]],
                ),
            )

        w_sb = {}
        tabs = {}

        def load_w(name, ext):
            wb = xw.tile([P, KCH, CO], BF16, tag=f"w{name}", name=f"w{name}")
            nc.scalar.dma_start(
                out=wb[:],
                in_=bass.AP(tensor=ext, offset=0,
                            ap=[[KCH * CO, P], [1, KCH * CO]]),
            )
            w_sb["w" + name] = wb

        def load_tab(nm):
            t_ = consts.tile([P, T], BF16, tag=nm)
            nc.scalar.dma_start(out=t_[:], in_=tab_ext[nm][:, :])
            tabs[nm] = t_

        # Act ring: weights + tables; SP ring: x.  The startup-critical
        # set (wq, wk, xq h0, xkv h0) is split across both rings.
        load_w("q", wq_ext)
        load_w("k", wk_ext)
        load_x("q", xq_ext, 0)
        load_x("kv", xkv_ext, 0)
        load_tab("cosq")
        load_tab("sinq")
        load_w("v", wv_ext)
        load_x("q", xq_ext, 1)
        load_x("kv", xkv_ext, 1)
        load_tab("cosk")
        load_tab("sink")

        bias_cols = {}
        for nm in ("bq", "bk", "bqs", "bks"):
            for ct in range(NCT):
                t_ = consts.tile([P, 1], F32, tag=f"{nm}{ct}")
                nc.gpsimd.dma_start(
                    out=t_[:], in_=bias_ext[nm][ct * P : (ct + 1) * P, :]
                )
                bias_cols[(nm, ct)] = t_
        bvb_sb = consts.tile([P, CO], F32, tag="bvb")
        nc.gpsimd.dma_start(
            out=bvb_sb[:],
            in_=bass.AP(tensor=bias_ext["bv"], offset=0, ap=[[0, P], [1, CO]]),
        )
        # persistent rotated q/k and v tiles
        rot_sb = {}
        for name in ("q", "k"):
            for ct in range(NCT):
                for hf in range(XH):
                    rot_sb[(name, ct, hf)] = persist.tile(
                        [P, TH], BF16, tag=f"r{name}{ct}{hf}",
                        name=f"r{name}{ct}{hf}"
                    )
        v_sb = [
            persist.tile([P, HEADS_PER_CORE, HEAD_DIM + 1], BF16,
                         tag=f"v{st}", name=f"v{st}")
            for st in range(NST)
        ]

        stack_p = ExitStack()
        projp = stack_p.enter_context(
            tc.tile_pool(name="projp", bufs=2, space="PSUM")
        )

        # ---------------- q/k projection + rotary (one t-half) ----------
        def proj_group(name, xsrc, ct, half):
            dst = rot_sb[(name, ct, half)]
            cos_t = tabs["cos" + name]
            sin_t = tabs["sin" + name]
            ps = projp.tile([P, TH], F32, tag="pj",
                            name=f"pj{name}{ct}_{half}")
            for k in range(KCH):
                for i in range(2):
                    nc.tensor.matmul(
                        ps[:, i * TCH : (i + 1) * TCH],
                        w_sb["w" + name][:, k, ct * P : (ct + 1) * P],
                        x_sb[(xsrc, half)][:, k, i * TCH : (i + 1) * TCH],
                        start=(k == 0),
                        stop=(k == KCH - 1),
                    )
            for i in range(2):
                tsl = slice(half * TH + i * TCH, half * TH + (i + 1) * TCH)
                psl = slice(i * TCH, (i + 1) * TCH)
                dsl = psl
                # rq = (q+b)*cos + (swap(q)+swap(b))*sin_signed
                qsb = projtmp.tile([P, TCH], BF16, tag="qsb",
                                   name=f"qsb{name}{ct}{half}{i}")
                nc.vector.tensor_scalar_add(
                    qsb[:], ps[:, psl], bias_cols[("b" + name, ct)][:]
                )
                qsw = projtmp.tile([P, TCH], F32, tag="qsw",
                                   name=f"qsw{name}{ct}{half}{i}")
                nc.vector.stream_shuffle(qsw[:], ps[:, psl], _SWAP_MASK)
                t2 = projtmp.tile([P, TCH], BF16, tag="rot2",
                                  name=f"t2{name}{ct}{half}{i}")
                nc.vector.scalar_tensor_tensor(
                    t2[:], qsw[:], bias_cols[("b" + name + "s", ct)][:],
                    sin_t[:, tsl], op0=Add, op1=Mul,
                )
                nc.vector.tensor_mul(dst[:, dsl], qsb[:], cos_t[:, tsl])
                nc.vector.tensor_add(dst[:, dsl], dst[:, dsl], t2[:])

        # pair-0 projections (DMA-paced; emission order matches DMA
        # arrival so the in-order PE queue never head-of-line blocks)
        proj_group("q", "q", 0, 0)
        proj_group("k", "kv", 0, 0)
        proj_group("q", "q", 0, 1)
        proj_group("k", "kv", 0, 1)

        # v projection ([128,256] fits a projp slot); emitted after the
        # it0 scores so it backfills PE idle instead of preempting them
        def emit_v_proj(sts):
            for st in sts:
                vt = v_sb[st]
                psv = projp.tile([P, CO], F32, tag="pj", name=f"psv{st}")
                half, col = divmod(st * P, TH)
                for k in range(KCH):
                    nc.tensor.matmul(
                        psv[:],
                        x_sb[("kv", half)][:, k, col : col + P],
                        w_sb["wv"][:, k, :],
                        start=(k == 0),
                        stop=(k == KCH - 1),
                    )
                nc.vector.tensor_add(
                    vt[:, :, 0:HEAD_DIM],
                    psv[:].rearrange("p (h d) -> p h d", h=HEADS_PER_CORE),
                    bvb_sb[:].rearrange("p (h d) -> p h d", h=HEADS_PER_CORE),
                )
                nc.vector.memset(vt[:, :, HEAD_DIM : HEAD_DIM + 1], 1.0)

        # ---------------- attention ----------------
        ITERS = [(p_, t_) for p_ in range(NCT) for t_ in range(2)]

        def scores_exp(it, st):
            pair, th = ITERS[it]
            rk = rot_sb[("k", pair, st // 8)]
            rq = rot_sb[("q", pair, th)]
            so = (st % 8) * P
            pss = [
                scp.tile([P, TH], F32, tag="sc", name=f"sc{it}_{st}_{h}")
                for h in range(2)
            ]
            # wave order: per tcc, all 4 (h, sh) quadrant MMs back-to-back
            # (disjoint row+col groups, 2 streams) -> concurrent execution
            for tcc in range(2):
                psl = slice(tcc * TCH, (tcc + 1) * TCH)
                for h in range(2):
                    for sh in range(2):
                        nc.tensor.matmul(
                            pss[h][sh * 64 : (sh + 1) * 64, psl],
                            rk[h * 64 : (h + 1) * 64,
                               so + sh * 64 : so + (sh + 1) * 64],
                            rq[h * 64 : (h + 1) * 64, psl],
                            start=True, stop=True,
                            tile_position=(h * 64, sh * 64),
                        )
            etiles = []
            for h in range(2):
                e = expp.tile([P, TH], BF16, tag="exp", name=f"e{it}_{st}_{h}")
                nc.scalar.activation(e[:], pss[h][:], ExpF, scale=SCALE)
                etiles.append(e)
            return etiles

        def av_mms(it, st, vps, etiles):
            pair, th = ITERS[it]
            for sub in range(2):
                h = pair * 2 + sub
                e = etiles[st][sub]
                for tcc in range(2):
                    psl = slice(tcc * TCH, (tcc + 1) * TCH)
                    nc.tensor.matmul(
                        vps[sub][:, psl],
                        v_sb[st][:, h, :],
                        e[:, psl],
                        start=(st == 0),
                        stop=(st == NST - 1),
                    )

        def epilogue(it, vps):
            pair, th = ITERS[it]
            for sub in range(2):
                h = pair * 2 + sub
                vcp = outp.tile([HEAD_DIM + 1, TH], F32, tag="vcp",
                                name=f"vcp{it}_{sub}")
                nc.vector.tensor_copy(vcp[:], vps[sub][:])
                dn = smallp.tile([1, TH], F32, tag="dn",
                                 name=f"dn{it}_{sub}")
                nc.sync.dma_start(
                    out=dn[:], in_=vcp[HEAD_DIM : HEAD_DIM + 1, :]
                )
                nc.vector.reciprocal_approx_fast(out=dn[:], in_=dn[:])
                recb = smallp.tile([1, TH], BF16, tag="recb",
                                   name=f"recb{it}_{sub}")
                nc.vector.tensor_copy(recb[:], dn[:])
                # broadcast 1->64 partitions on GpSimd (keeps PE queue free)
                rcb = smallp.tile([HEAD_DIM, TH], BF16, tag="rcb",
                                  name=f"rcb{it}_{sub}")
                for j in range(2):
                    nc.gpsimd.partition_broadcast(
                        rcb[:, j * TCH : (j + 1) * TCH],
                        recb[:, j * TCH : (j + 1) * TCH],
                        channels=HEAD_DIM,
                    )
                nc.vector.tensor_mul(
                    vcp[0:HEAD_DIM, :], vcp[0:HEAD_DIM, :], rcb[:]
                )
                nc.sync.dma_start(
                    out=out_ext[h * HEAD_DIM : (h + 1) * HEAD_DIM,
                                th * TH : (th + 1) * TH],
                    in_=vcp[0:HEAD_DIM, :],
                )

        emit_v_proj(range(0, 8))

        # window it0: scores only (projections backfill PE idle)
        et = {0: [scores_exp(0, st) for st in range(NST)]}

        # second half of v + pair-1 projections backfill windows it0/it1
        emit_v_proj(range(8, NST))
        proj_group("q", "q", 1, 0)
        proj_group("k", "kv", 1, 0)
        proj_group("q", "q", 1, 1)
        proj_group("k", "kv", 1, 1)
        stack_p.close()

        psva = stack_all.enter_context(
            tc.tile_pool(name="psva", bufs=2, space="PSUM")
        )

        def new_vps(it):
            return [
                psva.tile([HEAD_DIM + 1, TH], F32, tag="va",
                          name=f"vacc{it}_{s}")
                for s in range(2)
            ]

        # windows it1..it3: scores(it) + AV(it-1); then AV(it3) + epilogues
        vps = {}
        for it in range(1, 4):
            vps[it - 1] = new_vps(it - 1)
            et[it] = []
            for st in range(NST):
                et[it].append(scores_exp(it, st))
                av_mms(it - 1, st, vps[it - 1], et[it - 1])
            epilogue(it - 1, vps[it - 1])
        vps[3] = new_vps(3)
        for sub in range(2):
            pair, th = ITERS[3]
            h = pair * 2 + sub
            for st in range(NST):
                for tcc in range(2):
                    psl = slice(tcc * TCH, (tcc + 1) * TCH)
                    nc.tensor.matmul(
                        vps[3][sub][:, psl],
                        v_sb[st][:, h, :],
                        et[3][st][sub][:, psl],
                        start=(st == 0),
                        stop=(st == NST - 1),
                    )
        epilogue(3, vps[3])

        stack_all.close()
    nc.finalize()
    return nc


def make_in_maps(x_q, x_kv, q_positions, kv_positions, Wq, bq, Wk, bk, Wv, bv):
    x_q = np.asarray(x_q, np.float32)
    x_kv = np.asarray(x_kv, np.float32)
    q_positions = np.asarray(q_positions, np.int32)
    kv_positions = np.asarray(kv_positions, np.int32)
    Wq, Wk, Wv = (np.asarray(w, np.float32) for w in (Wq, Wk, Wv))
    bq, bk, bv = (np.asarray(b, np.float32) for b in (bq, bk, bv))

    xqT = [np.ascontiguousarray(x_q[b_].T).astype(NPBF16) for b_ in range(B)]
    xkvT = [np.ascontiguousarray(x_kv[b_].T).astype(NPBF16) for b_ in range(B)]
    tabs = []
    for b_ in range(B):
        cq, sq = _rot_tables(q_positions[b_])
        ck, sk = _rot_tables(kv_positions[b_])
        tabs.append((cq, sq, ck, sk))

    # pair-swapped bias vectors for the rotary shuffle path (swap within
    # rotated channels of each 64-channel head slot; identity elsewhere --
    # non-rotated channels multiply a zero sin so identity is harmless)
    swap_idx = np.arange(C)
    r = swap_idx % HEAD_DIM < N_ROT
    swap_idx[r] = swap_idx[r] ^ 1
    bqs_full = bq[swap_idx]
    bks_full = bk[swap_idx]

    in_maps = []
    for core in range(N_CORES):
        b_, hg = divmod(core, N_CORES // B)
        hsl = slice(hg * CO, (hg + 1) * CO)
        cq, sq, ck, sk = tabs[b_]
        def prepack(wT):  # [C, n] -> [P, KCH, n] partition-major
            n = wT.shape[1]
            return np.ascontiguousarray(
                wT.reshape(KCH, P, n).transpose(1, 0, 2)
            )

        m = {
            "wqT": prepack(Wq[hsl].T.astype(NPBF16)),
            "wkT": prepack(Wk[hsl].T.astype(NPBF16)),
            "wvT": prepack(Wv[hsl].T.astype(NPBF16)),
            "bq": np.ascontiguousarray(bq[hsl][:, None]),
            "bk": np.ascontiguousarray(bk[hsl][:, None]),
            "bqs": np.ascontiguousarray(bqs_full[hsl][:, None]),
            "bks": np.ascontiguousarray(bks_full[hsl][:, None]),
            "bv": np.ascontiguousarray(bv[hsl][:, None]),
            "cosq": cq, "sinq": sq, "cosk": ck, "sink": sk,
        }
        for h in range(XH):
            m[f"xqT{h}"] = prepack(xqT[b_][:, h * TH : (h + 1) * TH])
            m[f"xkvT{h}"] = prepack(xkvT[b_][:, h * TH : (h + 1) * TH])
        in_maps.append(m)
    return in_maps


_CACHED = {}


def kernel(x_q, x_kv, q_positions, kv_positions, Wq, bq, Wk, bk, Wv, bv):
    in_maps = make_in_maps(
        x_q, x_kv, q_positions, kv_positions, Wq, bq, Wk, bk, Wv, bv
    )
    if "nc" not in _CACHED:
        _CACHED["nc"] = build_bass()
    nc = _CACHED["nc"]

    res = run_bass_kernel_spmd(nc, in_maps, core_ids=list(range(N_CORES)))
    out = np.empty((B, T, C), np.float32)
    for core in range(N_CORES):
        b_, hg = divmod(core, N_CORES // B)
        out[b_, :, hg * CO : (hg + 1) * CO] = res.results[core]["out"].T
    return out


# revision 28
# speedup vs baseline: 1.1176x; 1.0344x over previous
"""
Bass/Trainium2 kernel for nn_Attention_72456098284196.

Attention module: QKV projections + partial rotary (first 32 of 64 head
channels, all heads) + softmax attention.  B=2, T=2048, C=1024, H=16, D=64.

Sharding: 8 NeuronCores = 2 batches x 4 head-groups (4 heads each).
Pure tensor/batch parallel -> no collectives; host slices inputs and
concatenates outputs.

Host prep: x / W are cast to bf16 and transposed; the rotary cos/sin
tables are precomputed from the position inputs ([128, T] bf16, one row
per channel of a 2-head c-tile; the splice sign is folded into the sin
table so the device-side splice is a plain partition pair-swap).

Device-side dataflow (per core, matmuls bf16 with fp32 PSUM accum):
  DMA: few large 128-partition transfers (x as two 2MB t-halves per
       tensor) so pair-0 projections start ~6us in.
  qT[c,t] = WqT.T @ xqT   (c = 4 heads x 64 ch, two 128-partition c-tiles)
  rotary:  rq = (q+b)*cos + (shuffle(q)+swap(b))*sin_signed on DVE
           (stream_shuffle pair-swap; passthrough channels cos=1,sin=0)
  scores^T[s,t] = rkT.T @ rqT per head, 64x64 PE tiling (two s-halves to
           disjoint col groups stream concurrently)
  expT = Exp(scores^T / 8) on ScalarE -> bf16.  ScalarE is the pacing
           engine (~1.14us per [128,1024] tile, 128 tiles = ~145us); the
           schedule keeps it fed from ~15us to the end:
             window it0: scores(it0) + q0/k0/v/q1/k1 projections backfill
             window itN: scores(itN) + AV(itN-1) backfill
  outT[d,t] accum over s of [v | 1].T @ expT  (M=65: row 64 accumulates
           the softmax denominator for free)
  normalize: recip(denominator row) via fast approx, broadcast 1->64
           partitions via a bf16 PE outer product, multiply, DMA out.
"""

import math
import sys

import numpy as np

if "/opt/trn_rl_repo" not in sys.path:
    sys.path.insert(0, "/opt/trn_rl_repo")

import concourse.bass as bass  # noqa: E402
import concourse.mybir as mybir  # noqa: E402
import concourse.tile as tile  # noqa: E402
from concourse import bacc  # noqa: E402
from concourse.bass_utils import run_bass_kernel_spmd  # noqa: E402

B, T, C = 2, 2048, 1024
NUM_HEADS = 16
HEAD_DIM = 64
N_CORES = 8
HEADS_PER_CORE = NUM_HEADS // (N_CORES // B)  # 4
CO = HEADS_PER_CORE * HEAD_DIM  # 256 out channels per core
N_ROT = 32  # rotated channels per head
MAX_WAVELENGTH = 8192.0

F32 = mybir.dt.float32
BF16 = mybir.dt.bfloat16
NPBF16 = mybir.dt.np(BF16)

P = 128  # partitions
TCH = 512  # matmul N chunk (1 PSUM bank)
KCH = C // P  # 8 contraction chunks
NCT = CO // P  # 2 c-tiles (each = 2 heads x 64)
NST = T // P  # 16 s tiles
SCALE = 1.0 / math.sqrt(HEAD_DIM)
TH = 1024  # attention t-half width / x DMA half width
XH = 2


def _inv_freq() -> np.ndarray:
    """[32] inverse frequencies (pairs repeated), matching the reference."""
    num_bands = N_ROT // 2  # 16
    freq = MAX_WAVELENGTH ** (
        2.0 / N_ROT * np.linspace(0.0, num_bands, num_bands, dtype=np.float64)
    )
    return np.repeat(1.0 / freq, 2)  # [32]


def _rot_tables(pos: np.ndarray) -> tuple[np.ndarray, np.ndarray]:
    """cos/sin tables [128, T] bf16 for a 2-head c-tile.

    Rows r in [0,32) and [64,96): rotary channels (cos/sin of pos*invf);
    other rows: cos=1, sin=0 (passthrough).  The splice negation is
    folded into sin: even channels get -sin so that
    rq = q*cos + pairswap(q)*sin_signed.
    """
    inv = _inv_freq()  # [32]
    rad = pos.astype(np.float64)[None, :] * inv[:, None]  # [32, T]
    cos32 = np.cos(rad)
    sin32 = np.sin(rad)
    sign = np.where(np.arange(N_ROT) % 2 == 0, -1.0, 1.0)[:, None]
    sin32 = sin32 * sign
    cos = np.zeros((P, pos.shape[0]), np.float64)
    sin = np.zeros((P, pos.shape[0]), np.float64)
    for o in (0, 64):
        cos[o : o + N_ROT] = cos32
        sin[o : o + N_ROT] = sin32
        cos[o + N_ROT : o + 64] = 1.0
    return cos.astype(NPBF16), sin.astype(NPBF16)


_SWAP_MASK = [i ^ 1 for i in range(32)]  # pair swap within each 32-quadrant


def build_bass() -> bass.Bass:
    nc = bacc.Bacc()

    # x / w are host-prepacked partition-major ([P, KCH, ...]) so each
    # DMA moves 128 x 16KB contiguous per-partition runs.
    xq_ext = [
        nc.declare_dram_parameter(f"xqT{h}", [P, KCH, TH], BF16, isOutput=False)
        for h in range(XH)
    ]
    xkv_ext = [
        nc.declare_dram_parameter(f"xkvT{h}", [P, KCH, TH], BF16, isOutput=False)
        for h in range(XH)
    ]
    wq_ext = nc.declare_dram_parameter("wqT", [P, KCH, CO], BF16, isOutput=False)
    wk_ext = nc.declare_dram_parameter("wkT", [P, KCH, CO], BF16, isOutput=False)
    wv_ext = nc.declare_dram_parameter("wvT", [P, KCH, CO], BF16, isOutput=False)
    bias_ext = {}
    for nm in ("bq", "bk", "bqs", "bks", "bv"):
        bias_ext[nm] = nc.declare_dram_parameter(nm, [CO, 1], F32, isOutput=False)
    tab_ext = {}
    for nm in ("cosq", "sinq", "cosk", "sink"):
        tab_ext[nm] = nc.declare_dram_parameter(nm, [P, T], BF16, isOutput=False)
    out_ext = nc.declare_dram_parameter("out", [CO, T], F32, isOutput=True)

    ExpF = mybir.ActivationFunctionType.Exp
    Mul = mybir.AluOpType.mult
    Add = mybir.AluOpType.add

    with tile.TileContext(nc) as tc:
        from contextlib import ExitStack

        stack_all = ExitStack()
        consts = stack_all.enter_context(tc.tile_pool(name="consts", bufs=1))
        persist = stack_all.enter_context(tc.tile_pool(name="persist", bufs=1))
        xw = stack_all.enter_context(tc.tile_pool(name="xw", bufs=1))
        projtmp = stack_all.enter_context(tc.tile_pool(name="projtmp", bufs=2))
        scp = stack_all.enter_context(
            tc.tile_pool(name="scp", bufs=2, space="PSUM")
        )
        expp = stack_all.enter_context(tc.tile_pool(name="expp", bufs=31))
        outp = stack_all.enter_context(tc.tile_pool(name="outp", bufs=2))
        smallp = stack_all.enter_context(tc.tile_pool(name="small", bufs=1))

        # ---------------- input DMAs: few, large, 128-partition ----------
        # SP ring: xq h0, xkv h0, xq h1, xkv h1 (2MB each).
        # Act ring: wq, cos/sin(q), wk, wv, cos/sin(k).
        # GpSimd ring: bias columns.
        x_sb = {}
        for name in ("q", "kv"):
            for h in range(XH):
                x_sb[(name, h)] = xw.tile([P, KCH, TH], BF16, tag=f"x{name}{h}",
                                          name=f"x{name}{h}")

        def load_x(name, exts, h):
            # split each 2MB half across both HWDGE rings (1MB each)
            half_elems = (KCH // 2) * TH
            nc.sync.dma_start(
                out=x_sb[(name, h)][:, 0 : KCH // 2, :],
                in_=bass.AP(tensor=exts[h], offset=0,
                            ap=[[KCH * TH, P], [1, half_elems]]),
            )
            nc.scalar.dma_start(
                out=x_sb[(name, h)][:, KCH // 2 : KCH, :],
                in_=bass.AP(tensor=exts[h], offset=half_elems,
                            ap=[[KCH * TH, P], [1, half_elems]]),
            )

        w_sb = {}
        tabs = {}

        def load_w(name, ext):
            wb = xw.tile([P, KCH, CO], BF16, tag=f"w{name}", name=f"w{name}")
            nc.scalar.dma_start(
                out=wb[:],
                in_=bass.AP(tensor=ext, offset=0,
                            ap=[[KCH * CO, P], [1, KCH * CO]]),
            )
            w_sb["w" + name] = wb

        def load_tab(nm):
            t_ = consts.tile([P, T], BF16, tag=nm)
            nc.scalar.dma_start(out=t_[:], in_=tab_ext[nm][:, :])
            tabs[nm] = t_

        # startup-critical order: weights, xq h0, q-tables, xkv h0,
        # k-tables, wv, then the second halves.
        load_w("q", wq_ext)
        load_w("k", wk_ext)
        load_x("q", xq_ext, 0)
        load_tab("cosq")
        load_tab("sinq")
        load_x("kv", xkv_ext, 0)
        load_tab("cosk")
        load_tab("sink")
        load_w("v", wv_ext)
        load_x("q", xq_ext, 1)
        load_x("kv", xkv_ext, 1)

        bias_cols = {}
        for nm in ("bq", "bk", "bqs", "bks"):
            for ct in range(NCT):
                t_ = consts.tile([P, 1], F32, tag=f"{nm}{ct}")
                nc.gpsimd.dma_start(
                    out=t_[:], in_=bias_ext[nm][ct * P : (ct + 1) * P, :]
                )
                bias_cols[(nm, ct)] = t_
        bvb_sb = consts.tile([P, CO], F32, tag="bvb")
        nc.gpsimd.dma_start(
            out=bvb_sb[:],
            in_=bass.AP(tensor=bias_ext["bv"], offset=0, ap=[[0, P], [1, CO]]),
        )
        # persistent rotated q/k and v tiles
        rot_sb = {}
        for name in ("q", "k"):
            for ct in range(NCT):
                for hf in range(XH):
                    rot_sb[(name, ct, hf)] = persist.tile(
                        [P, TH], BF16, tag=f"r{name}{ct}{hf}",
                        name=f"r{name}{ct}{hf}"
                    )
        v_sb = [
            persist.tile([P, HEADS_PER_CORE, HEAD_DIM + 1], BF16,
                         tag=f"v{st}", name=f"v{st}")
            for st in range(NST)
        ]

        stack_p = ExitStack()
        projp = stack_p.enter_context(
            tc.tile_pool(name="projp", bufs=4, space="PSUM")
        )

        # ---------------- q/k projection + rotary (one t-half) ----------
        def proj_group(name, xsrc, ct, half):
            dst = rot_sb[(name, ct, half)]
            cos_t = tabs["cos" + name]
            sin_t = tabs["sin" + name]
            pss = [
                projp.tile([P, TCH], F32, tag="pj",
                           name=f"pj{name}{ct}_{half}_{i}")
                for i in range(2)
            ]
            for k in range(KCH):
                for i in range(2):
                    nc.tensor.matmul(
                        pss[i][:],
                        w_sb["w" + name][:, k, ct * P : (ct + 1) * P],
                        x_sb[(xsrc, half)][:, k, i * TCH : (i + 1) * TCH],
                        start=(k == 0),
                        stop=(k == KCH - 1),
                    )
            for i in range(2):
                ps = pss[i]
                tsl = slice(half * TH + i * TCH, half * TH + (i + 1) * TCH)
                psl = slice(0, TCH)
                dsl = slice(i * TCH, (i + 1) * TCH)
                # rq = (q+b)*cos + (swap(q)+swap(b))*sin_signed
                qsb = projtmp.tile([P, TCH], BF16, tag="qsb",
                                   name=f"qsb{name}{ct}{half}{i}")
                nc.vector.tensor_scalar_add(
                    qsb[:], ps[:, psl], bias_cols[("b" + name, ct)][:]
                )
                qsw = projtmp.tile([P, TCH], F32, tag="qsw",
                                   name=f"qsw{name}{ct}{half}{i}")
                nc.vector.stream_shuffle(qsw[:], ps[:, psl], _SWAP_MASK)
                t2 = projtmp.tile([P, TCH], BF16, tag="rot2",
                                  name=f"t2{name}{ct}{half}{i}")
                nc.vector.scalar_tensor_tensor(
                    t2[:], qsw[:], bias_cols[("b" + name + "s", ct)][:],
                    sin_t[:, tsl], op0=Add, op1=Mul,
                )
                nc.vector.tensor_mul(dst[:, dsl], qsb[:], cos_t[:, tsl])
                nc.vector.tensor_add(dst[:, dsl], dst[:, dsl], t2[:])

        # pair-0 projections (DMA-paced; emission order matches DMA
        # arrival so the in-order PE queue never head-of-line blocks)
        proj_group("q", "q", 0, 0)
        proj_group("k", "kv", 0, 0)
        proj_group("q", "q", 0, 1)
        proj_group("k", "kv", 0, 1)

        # v projection ([128,256] fits a projp slot); emitted after the
        # it0 scores so it backfills PE idle instead of preempting them
        def emit_v_proj(sts):
            for st in sts:
                vt = v_sb[st]
                psv = projp.tile([P, CO], F32, tag="pj", name=f"psv{st}")
                half, col = divmod(st * P, TH)
                for k in range(KCH):
                    nc.tensor.matmul(
                        psv[:],
                        x_sb[("kv", half)][:, k, col : col + P],
                        w_sb["wv"][:, k, :],
                        start=(k == 0),
                        stop=(k == KCH - 1),
                    )
                nc.vector.tensor_add(
                    vt[:, :, 0:HEAD_DIM],
                    psv[:].rearrange("p (h d) -> p h d", h=HEADS_PER_CORE),
                    bvb_sb[:].rearrange("p (h d) -> p h d", h=HEADS_PER_CORE),
                )
                nc.vector.memset(vt[:, :, HEAD_DIM : HEAD_DIM + 1], 1.0)

        # ---------------- attention ----------------
        ITERS = [(p_, t_) for p_ in range(NCT) for t_ in range(2)]

        def scores_exp(it, st):
            pair, th = ITERS[it]
            rk = rot_sb[("k", pair, st // 8)]
            rq = rot_sb[("q", pair, th)]
            so = (st % 8) * P
            pss = [
                scp.tile([P, TH], F32, tag="sc", name=f"sc{it}_{st}_{h}")
                for h in range(2)
            ]
            # wave order: per tcc, all 4 (h, sh) quadrant MMs back-to-back
            # (disjoint row+col groups, 2 streams) -> concurrent execution
            for tcc in range(2):
                psl = slice(tcc * TCH, (tcc + 1) * TCH)
                for h in range(2):
                    for sh in range(2):
                        nc.tensor.matmul(
                            pss[h][sh * 64 : (sh + 1) * 64, psl],
                            rk[h * 64 : (h + 1) * 64,
                               so + sh * 64 : so + (sh + 1) * 64],
                            rq[h * 64 : (h + 1) * 64, psl],
                            start=True, stop=True,
                            tile_position=(h * 64, sh * 64),
                        )
            etiles = []
            for h in range(2):
                e = expp.tile([P, TH], BF16, tag="exp", name=f"e{it}_{st}_{h}")
                nc.scalar.activation(e[:], pss[h][:], ExpF, scale=SCALE)
                etiles.append(e)
            return etiles

        def av_mms(it, st, vps, etiles):
            pair, th = ITERS[it]
            for sub in range(2):
                h = pair * 2 + sub
                e = etiles[st][sub]
                for tcc in range(2):
                    psl = slice(tcc * TCH, (tcc + 1) * TCH)
                    nc.tensor.matmul(
                        vps[sub][:, psl],
                        v_sb[st][:, h, :],
                        e[:, psl],
                        start=(st == 0),
                        stop=(st == NST - 1),
                    )

        def epilogue(it, vps):
            pair, th = ITERS[it]
            for sub in range(2):
                h = pair * 2 + sub
                vcp = outp.tile([HEAD_DIM + 1, TH], F32, tag="vcp",
                                name=f"vcp{it}_{sub}")
                nc.vector.tensor_copy(vcp[:], vps[sub][:])
                dn = smallp.tile([1, TH], F32, tag="dn",
                                 name=f"dn{it}_{sub}")
                nc.sync.dma_start(
                    out=dn[:], in_=vcp[HEAD_DIM : HEAD_DIM + 1, :]
                )
                nc.vector.reciprocal_approx_fast(out=dn[:], in_=dn[:])
                recb = smallp.tile([1, TH], BF16, tag="recb",
                                   name=f"recb{it}_{sub}")
                nc.vector.tensor_copy(recb[:], dn[:])
                # broadcast 1->64 partitions on GpSimd (keeps PE queue free)
                rcb = smallp.tile([HEAD_DIM, TH], BF16, tag="rcb",
                                  name=f"rcb{it}_{sub}")
                for j in range(2):
                    nc.gpsimd.partition_broadcast(
                        rcb[:, j * TCH : (j + 1) * TCH],
                        recb[:, j * TCH : (j + 1) * TCH],
                        channels=HEAD_DIM,
                    )
                nc.vector.tensor_mul(
                    vcp[0:HEAD_DIM, :], vcp[0:HEAD_DIM, :], rcb[:]
                )
                nc.sync.dma_start(
                    out=out_ext[h * HEAD_DIM : (h + 1) * HEAD_DIM,
                                th * TH : (th + 1) * TH],
                    in_=vcp[0:HEAD_DIM, :],
                )

        emit_v_proj(range(0, 8))

        # window it0: scores only (projections backfill PE idle)
        et = {0: [scores_exp(0, st) for st in range(NST)]}

        # second half of v + pair-1 projections backfill windows it0/it1
        emit_v_proj(range(8, NST))
        proj_group("q", "q", 1, 0)
        proj_group("k", "kv", 1, 0)
        proj_group("q", "q", 1, 1)
        proj_group("k", "kv", 1, 1)
        stack_p.close()

        psva = stack_all.enter_context(
            tc.tile_pool(name="psva", bufs=2, space="PSUM")
        )

        def new_vps(it):
            return [
                psva.tile([HEAD_DIM + 1, TH], F32, tag="va",
                          name=f"vacc{it}_{s}")
                for s in range(2)
            ]

        # windows it1..it3: scores(it) + AV(it-1); then AV(it3) + epilogues
        vps = {}
        for it in range(1, 4):
            vps[it - 1] = new_vps(it - 1)
            et[it] = []
            for st in range(NST):
                et[it].append(scores_exp(it, st))
                av_mms(it - 1, st, vps[it - 1], et[it - 1])
            epilogue(it - 1, vps[it - 1])
        vps[3] = new_vps(3)
        for sub in range(2):
            pair, th = ITERS[3]
            h = pair * 2 + sub
            for st in range(NST):
                for tcc in range(2):
                    psl = slice(tcc * TCH, (tcc + 1) * TCH)
                    nc.tensor.matmul(
                        vps[3][sub][:, psl],
                        v_sb[st][:, h, :],
                        et[3][st][sub][:, psl],
                        start=(st == 0),
                        stop=(st == NST - 1),
                    )
        epilogue(3, vps[3])

        stack_all.close()
    nc.finalize()
    return nc


def make_in_maps(x_q, x_kv, q_positions, kv_positions, Wq, bq, Wk, bk, Wv, bv):
    x_q = np.asarray(x_q, np.float32)
    x_kv = np.asarray(x_kv, np.float32)
    q_positions = np.asarray(q_positions, np.int32)
    kv_positions = np.asarray(kv_positions, np.int32)
    Wq, Wk, Wv = (np.asarray(w, np.float32) for w in (Wq, Wk, Wv))
    bq, bk, bv = (np.asarray(b, np.float32) for b in (bq, bk, bv))

    xqT = [np.ascontiguousarray(x_q[b_].T).astype(NPBF16) for b_ in range(B)]
    xkvT = [np.ascontiguousarray(x_kv[b_].T).astype(NPBF16) for b_ in range(B)]
    tabs = []
    for b_ in range(B):
        cq, sq = _rot_tables(q_positions[b_])
        ck, sk = _rot_tables(kv_positions[b_])
        tabs.append((cq, sq, ck, sk))

    # pair-swapped bias vectors for the rotary shuffle path (swap within
    # rotated channels of each 64-channel head slot; identity elsewhere --
    # non-rotated channels multiply a zero sin so identity is harmless)
    swap_idx = np.arange(C)
    r = swap_idx % HEAD_DIM < N_ROT
    swap_idx[r] = swap_idx[r] ^ 1
    bqs_full = bq[swap_idx]
    bks_full = bk[swap_idx]

    in_maps = []
    for core in range(N_CORES):
        b_, hg = divmod(core, N_CORES // B)
        hsl = slice(hg * CO, (hg + 1) * CO)
        cq, sq, ck, sk = tabs[b_]
        def prepack(wT):  # [C, n] -> [P, KCH, n] partition-major
            n = wT.shape[1]
            return np.ascontiguousarray(
                wT.reshape(KCH, P, n).transpose(1, 0, 2)
            )

        m = {
            "wqT": prepack(Wq[hsl].T.astype(NPBF16)),
            "wkT": prepack(Wk[hsl].T.astype(NPBF16)),
            "wvT": prepack(Wv[hsl].T.astype(NPBF16)),
            "bq": np.ascontiguousarray(bq[hsl][:, None]),
            "bk": np.ascontiguousarray(bk[hsl][:, None]),
            "bqs": np.ascontiguousarray(bqs_full[hsl][:, None]),
            "bks": np.ascontiguousarray(bks_full[hsl][:, None]),
            "bv": np.ascontiguousarray(bv[hsl][:, None]),
            "cosq": cq, "sinq": sq, "cosk": ck, "sink": sk,
        }
        for h in range(XH):
            m[f"xqT{h}"] = prepack(xqT[b_][:, h * TH : (h + 1) * TH])
            m[f"xkvT{h}"] = prepack(xkvT[b_][:, h * TH : (h + 1) * TH])
        in_maps.append(m)
    return in_maps


_CACHED = {}


def kernel(x_q, x_kv, q_positions, kv_positions, Wq, bq, Wk, bk, Wv, bv):
    in_maps = make_in_maps(
        x_q, x_kv, q_positions, kv_positions, Wq, bq, Wk, bk, Wv, bv
    )
    if "nc" not in _CACHED:
        _CACHED["nc"] = build_bass()
    nc = _CACHED["nc"]

    res = run_bass_kernel_spmd(nc, in_maps, core_ids=list(range(N_CORES)))
    out = np.empty((B, T, C), np.float32)
    for core in range(N_CORES):
        b_, hg = divmod(core, N_CORES // B)
        out[b_, :, hg * CO : (hg + 1) * CO] = res.results[core]["out"].T
    return out
